# revision 1
# baseline (speedup 1.0000x reference)
"""ABCNN1 attention kernel for 8 Trainium2 NeuronCores.

Reference computation (per batch b of 64, with L=512, D=1024):
    S  = X1 @ X2^T                          (512 x 512)
    A  = S / (|X1_rows| outer |X2_rows|)    cosine match-score
    a1 = A @ W1            a2 = A^T @ W2    (512 x 1024 each)
    attn1 = concat([x1, a1], axis=1)        attn2 = concat([x2, a2], axis=1)

Device strategy (data-parallel, 8 batches per core, no collectives):
  - Host pre-transposes x1/x2 to [b, D, L] so the contraction dim d lands
    on SBUF partitions with fully contiguous DMA.
  - Norms: nsq[l] = sum_d X^2 computed as ones^T @ (X.^2) row-vector
    matmuls, then sqrt + reciprocal; a 4KB DRAM bounce converts the
    [1, 1024] row into per-partition [128, 8] scale columns.
  - Normalization is folded into per-partition scalar multiplies:
      a2's lhsT = diag(r1) @ S   (scaled PSUM->SBUF copy)
      a1's lhsT = diag(r2) @ S^T (scaled PSUM->SBUF copy)
    and the remaining diagonal lands on the matmul *outputs* (also a
    per-partition scaled copy). No cross-partition broadcasts anywhere.
  - Matmuls run in float32r (full-rate on TensorE at N=512, ~1e-4 rel).
  - The concat halves of the outputs are just the inputs; they are
    assembled on host - the device computes and writes only a1/a2.
"""

import numpy as np

B, L, D = 64, 512, 1024
N_CORES = 8
BB = B // N_CORES        # batches per core
KT = D // 128            # contraction tiles (d)
LT = L // 128            # row tiles (l or m)
NT = D // 512            # output free-dim chunks

_CACHE = {}


def _build(bb):
    import concourse.mybir as mybir
    import concourse.tile as tile
    from concourse import bacc

    F32 = mybir.dt.float32
    F32R = mybir.dt.float32r
    BF16 = mybir.dt.bfloat16

    nc = bacc.Bacc("TRN2", target_bir_lowering=False, debug=False,
                   num_devices=N_CORES)
    x1t = nc.declare_dram_parameter("x1t", [bb, D, L], F32, isOutput=False)
    x2t = nc.declare_dram_parameter("x2t", [bb, D, L], F32, isOutput=False)
    w1 = nc.declare_dram_parameter("w1", [L, D], F32, isOutput=False)
    w2 = nc.declare_dram_parameter("w2", [L, D], F32, isOutput=False)
    out1 = nc.declare_dram_parameter("out1", [bb, L, D], F32, isOutput=True)
    out2 = nc.declare_dram_parameter("out2", [bb, L, D], F32, isOutput=True)

    with tile.TileContext(nc) as tc:
        with (
            tc.tile_pool(name="const", bufs=1) as constp,
            tc.tile_pool(name="xin", bufs=2) as xin,
            tc.tile_pool(name="sq", bufs=4) as sqp,
            tc.tile_pool(name="alhs", bufs=2) as alhsp,
            tc.tile_pool(name="aout", bufs=4) as aoutp,
            tc.tile_pool(name="small", bufs=2) as smallp,
            tc.tile_pool(name="dram", bufs=2, space="DRAM") as dramp,
            tc.tile_pool(name="ps_s", bufs=4, space="PSUM") as ps_s,
            tc.tile_pool(name="ps_nsq", bufs=2, space="PSUM") as ps_nsq,
            tc.tile_pool(name="ps_a", bufs=2, space="PSUM") as ps_a,
        ):
            # ---- persistent tiles -------------------------------------
            w1_sb = constp.tile([128, LT, D], F32R, tag="w1")
            w2_sb = constp.tile([128, LT, D], F32R, tag="w2")
            nc.gpsimd.dma_start(w1_sb[:], w1.rearrange("(j p) d -> p j d", p=128))
            nc.gpsimd.dma_start(w2_sb[:], w2.rearrange("(i p) d -> p i d", p=128))
            ones_sb = constp.tile([128, 1], BF16, tag="ones")
            nc.gpsimd.memset(ones_sb[:], 1.0)

            for b in range(bb):
                # ---- load inputs (f32 -> f32r casting DMA) ------------
                x1r = xin.tile([128, KT, L], F32R, tag="x1")
                x2r = xin.tile([128, KT, L], F32R, tag="x2")
                nc.gpsimd.dma_start(x1r[:], x1t[b].rearrange("(k p) l -> p k l", p=128))
                nc.gpsimd.dma_start(x2r[:], x2t[b].rearrange("(k p) l -> p k l", p=128))

                # ---- row-form squared norms ---------------------------
                nsq1 = ps_nsq.tile([1, L], F32, tag="nsq")
                nsq2 = ps_nsq.tile([1, L], F32, tag="nsq")
                for x_r, nsq in ((x1r, nsq1), (x2r, nsq2)):
                    for k in range(KT):
                        xsq = sqp.tile([128, L], BF16, tag="xsq")
                        nc.vector.tensor_mul(
                            xsq[:], x_r[:, k, :].bitcast(F32), x_r[:, k, :].bitcast(F32)
                        )
                        nc.tensor.matmul(nsq[:], ones_sb[:], xsq[:],
                                         start=(k == 0), stop=(k == KT - 1))

                # r = 1/sqrt(nsq), then DRAM-bounce into [128, 8] columns
                srow = smallp.tile([1, 2 * L], F32, tag="srow")
                nc.scalar.sqrt(srow[:, 0:L], nsq1[:])
                nc.scalar.sqrt(srow[:, L:2 * L], nsq2[:])
                rrow = smallp.tile([1, 2 * L], F32, tag="rrow")
                nc.vector.reciprocal(rrow[:], srow[:])
                r_dram = dramp.tile([1, 2 * L], F32, tag="rd")
                nc.sync.dma_start(r_dram[:], rrow[:])
                r_sb = smallp.tile([128, 2 * LT], F32, tag="rsb")
                nc.sync.dma_start(r_sb[:], r_dram.rearrange("o (c p) -> (o p) c", p=128))

                # ---- S = X1 @ X2^T and S^T, normalization folded ------
                # a2lhs[:, i, :] = r1-scaled S l-tile i   (lhsT for a2)
                # a1lhs[:, j, :] = r2-scaled S^T m-tile j (lhsT for a1)
                a2lhs = alhsp.tile([128, LT, L], F32R, tag="a2lhs")
                a1lhs = alhsp.tile([128, LT, L], F32R, tag="a1lhs")
                for i in range(LT):
                    s_ps = ps_s.tile([128, L], F32, tag="s")
                    for k in range(KT):
                        nc.tensor.matmul(s_ps[:], x1r[:, k, 128 * i:128 * (i + 1)],
                                         x2r[:, k, :], start=(k == 0), stop=(k == KT - 1))
                    nc.vector.tensor_scalar_mul(a2lhs[:, i, :], s_ps[:], r_sb[:, i:i + 1])
                for j in range(LT):
                    st_ps = ps_s.tile([128, L], F32, tag="s")
                    for k in range(KT):
                        nc.tensor.matmul(st_ps[:], x2r[:, k, 128 * j:128 * (j + 1)],
                                         x1r[:, k, :], start=(k == 0), stop=(k == KT - 1))
                    nc.vector.tensor_scalar_mul(a1lhs[:, j, :], st_ps[:],
                                                r_sb[:, LT + j:LT + j + 1])

                # ---- stage 2: a1 = (S D2^-1 W1) row-scaled by r1 ------
                import concourse.mybir as _mybir
                Copy = _mybir.ActivationFunctionType.Copy
                for i in range(LT):
                    a1_sb = aoutp.tile([128, D], F32, tag="aout")
                    for n in range(NT):
                        a1_ps = ps_a.tile([128, 512], F32, tag="a")
                        for jj in range(LT):
                            nc.tensor.matmul(
                                a1_ps[:], a1lhs[:, jj, 128 * i:128 * (i + 1)],
                                w1_sb[:, jj, 512 * n:512 * (n + 1)],
                                start=(jj == 0), stop=(jj == LT - 1))
                        nc.scalar.activation(a1_sb[:, 512 * n:512 * (n + 1)], a1_ps[:],
                                             Copy, scale=r_sb[:, i:i + 1])
                    nc.sync.dma_start(out1[b, 128 * i:128 * (i + 1), :], a1_sb[:])
                # ---- stage 2: a2 = (S^T D1^-1 W2) row-scaled by r2 ----
                for j in range(LT):
                    a2_sb = aoutp.tile([128, D], F32, tag="aout")
                    for n in range(NT):
                        a2_ps = ps_a.tile([128, 512], F32, tag="a")
                        for ii in range(LT):
                            nc.tensor.matmul(
                                a2_ps[:], a2lhs[:, ii, 128 * j:128 * (j + 1)],
                                w2_sb[:, ii, 512 * n:512 * (n + 1)],
                                start=(ii == 0), stop=(ii == LT - 1))
                        nc.vector.tensor_scalar_mul(a2_sb[:, 512 * n:512 * (n + 1)],
                                                    a2_ps[:], r_sb[:, LT + j:LT + j + 1])
                    nc.sync.dma_start(out2[b, 128 * j:128 * (j + 1), :], a2_sb[:])

    nc.compile()
    return nc


def _get_nc(bb=BB):
    if bb not in _CACHE:
        _CACHE[bb] = _build(bb)
    return _CACHE[bb]


def run_device(x1, x2, W1, W2, trace=False, bb=BB, n_batches=None):
    """Run the device part; returns (a1, a2) of shape (n, L, D) and the
    raw BassKernelResults (for exec_time_ns when trace=True)."""
    import concourse.bass_utils as bass_utils

    n = n_batches if n_batches is not None else bb * N_CORES
    x1 = np.ascontiguousarray(np.asarray(x1, dtype=np.float32).reshape(n, L, D)
                              .transpose(0, 2, 1))
    x2 = np.ascontiguousarray(np.asarray(x2, dtype=np.float32).reshape(n, L, D)
                              .transpose(0, 2, 1))
    W1 = np.ascontiguousarray(np.asarray(W1, dtype=np.float32))
    W2 = np.ascontiguousarray(np.asarray(W2, dtype=np.float32))

    nc = _get_nc(bb)
    in_maps = []
    for c in range(N_CORES):
        s = slice(c * bb, (c + 1) * bb)
        in_maps.append({"x1t": x1[s], "x2t": x2[s], "w1": W1, "w2": W2})
    res = bass_utils.run_bass_kernel_spmd(nc, in_maps, list(range(N_CORES)),
                                          trace=trace)
    a1 = np.concatenate([res.results[c]["out1"] for c in range(N_CORES)], axis=0)
    a2 = np.concatenate([res.results[c]["out2"] for c in range(N_CORES)], axis=0)
    return a1, a2, res


def kernel(x1, x2, W1, W2):
    x1 = np.asarray(x1, dtype=np.float32)
    x2 = np.asarray(x2, dtype=np.float32)
    a1, a2, _ = run_device(x1, x2, W1, W2, trace=False)
    attn1 = np.stack([x1.reshape(B, L, D), a1], axis=1)
    attn2 = np.stack([x2.reshape(B, L, D), a2], axis=1)
    return attn1, attn2


# revision 5
# speedup vs baseline: 1.3284x; 1.3284x over previous
"""ABCNN1 attention kernel for 8 Trainium2 NeuronCores.

Reference computation (per batch b of 64, with L=512, D=1024):
    S  = X1 @ X2^T                          (512 x 512)
    A  = S / (|X1_rows| outer |X2_rows|)    cosine match-score
    a1 = A @ W1            a2 = A^T @ W2    (512 x 1024 each)
    attn1 = concat([x1, a1], axis=1)        attn2 = concat([x2, a2], axis=1)

Device strategy (data-parallel, 8 batches per core, no collectives):
  - Host pre-transposes x1/x2 to [b, D, L] so the contraction dim d lands
    on SBUF partitions with fully contiguous DMA.
  - Norms: nsq[l] = sum_d X^2 computed as ones^T @ (X.^2) row-vector
    matmuls, then sqrt + reciprocal; a 4KB DRAM bounce converts the
    [1, 1024] row into per-partition [128, 8] scale columns.
  - Normalization is folded into per-partition scalar multiplies:
      a2's lhsT = diag(r1) @ S   (scaled PSUM->SBUF copy)
      a1's lhsT = diag(r2) @ S^T (scaled PSUM->SBUF copy)
    and the remaining diagonal lands on the matmul *outputs* (also a
    per-partition scaled copy). No cross-partition broadcasts anywhere.
  - Matmuls run in float32r (full-rate on TensorE at N=512, ~1e-4 rel).
  - The concat halves of the outputs are just the inputs; they are
    assembled on host - the device computes and writes only a1/a2.
"""

import numpy as np

B, L, D = 64, 512, 1024
N_CORES = 8
BB = B // N_CORES        # batches per core
KT = D // 128            # contraction tiles (d)
LT = L // 128            # row tiles (l or m)
NT = D // 512            # output free-dim chunks

_CACHE = {}


def _build(bb):
    import concourse.mybir as mybir
    import concourse.tile as tile
    from concourse import bacc

    F32 = mybir.dt.float32
    F32R = mybir.dt.float32r
    BF16 = mybir.dt.bfloat16

    nc = bacc.Bacc("TRN2", target_bir_lowering=False, debug=False,
                   num_devices=N_CORES)
    x1t = nc.declare_dram_parameter("x1t", [bb, D, L], F32, isOutput=False)
    x2t = nc.declare_dram_parameter("x2t", [bb, D, L], F32, isOutput=False)
    w1 = nc.declare_dram_parameter("w1", [L, D], F32, isOutput=False)
    w2 = nc.declare_dram_parameter("w2", [L, D], F32, isOutput=False)
    out1 = nc.declare_dram_parameter("out1", [bb, L, D], F32, isOutput=True)
    out2 = nc.declare_dram_parameter("out2", [bb, L, D], F32, isOutput=True)

    with tile.TileContext(nc) as tc:
        with (
            tc.tile_pool(name="const", bufs=1) as constp,
            tc.tile_pool(name="xin", bufs=2) as xin,
            tc.tile_pool(name="sq", bufs=2) as sqp,
            tc.tile_pool(name="alhs", bufs=2) as alhsp,
            tc.tile_pool(name="aout", bufs=4) as aoutp,
            tc.tile_pool(name="small", bufs=2) as smallp,
            tc.tile_pool(name="dram", bufs=2, space="DRAM") as dramp,
            tc.tile_pool(name="ps_s", bufs=4, space="PSUM") as ps_s,
            tc.tile_pool(name="ps_nsq", bufs=2, space="PSUM") as ps_nsq,
            tc.tile_pool(name="ps_a", bufs=2, space="PSUM") as ps_a,
        ):
            # ---- persistent tiles -------------------------------------
            w1_sb = constp.tile([128, LT, D], F32R, tag="w1")
            w2_sb = constp.tile([128, LT, D], F32R, tag="w2")
            nc.gpsimd.dma_start(w1_sb[:], w1.rearrange("(j p) d -> p j d", p=128))
            nc.gpsimd.dma_start(w2_sb[:], w2.rearrange("(i p) d -> p i d", p=128))
            ones_sb = constp.tile([128, 1], BF16, tag="ones")
            nc.gpsimd.memset(ones_sb[:], 1.0)

            for b in range(bb):
                # ---- load inputs (f32 -> f32r casting DMA) ------------
                x1r = xin.tile([128, KT, L], F32R, tag="x1")
                x2r = xin.tile([128, KT, L], F32R, tag="x2")
                nc.gpsimd.dma_start(x1r[:], x1t[b].rearrange("(k p) l -> p k l", p=128))
                nc.gpsimd.dma_start(x2r[:], x2t[b].rearrange("(k p) l -> p k l", p=128))

                # ---- row-form squared norms ---------------------------
                # squares per k-tile (bf16), in-place pair-add tree, then a
                # single ones^T @ xsq_acc matmul per input -> nsq row [1, L]
                nsq1 = ps_nsq.tile([1, L], F32, tag="nsq")
                nsq2 = ps_nsq.tile([1, L], F32, tag="nsq")
                for x_r, nsq in ((x1r, nsq1), (x2r, nsq2)):
                    xsq = [sqp.tile([128, L], BF16, tag=f"xsq{k}", name=f"xsq{k}")
                           for k in range(KT)]
                    for k in range(KT):
                        nc.vector.tensor_mul(
                            xsq[k][:], x_r[:, k, :].bitcast(F32), x_r[:, k, :].bitcast(F32)
                        )
                    step = 1
                    while step < KT:
                        for k in range(0, KT, 2 * step):
                            nc.vector.tensor_add(xsq[k][:], xsq[k][:], xsq[k + step][:])
                        step *= 2
                    nc.tensor.matmul(nsq[:], ones_sb[:], xsq[0][:], start=True, stop=True)

                # sqrt doubles as the PSUM->SBUF copy (row form), then a
                # DRAM bounce scatters rows into per-partition [128, 8]
                # columns where the reciprocal is cheap.
                srow = smallp.tile([1, 2 * L], F32, tag="srow")
                nc.scalar.sqrt(srow[:, 0:L], nsq1[:])
                nc.scalar.sqrt(srow[:, L:2 * L], nsq2[:])
                r_dram = dramp.tile([1, 2 * L], F32, tag="rd")
                nc.sync.dma_start(r_dram[:], srow[:])
                rst_sb = smallp.tile([128, 2 * LT], F32, tag="rst")
                nc.sync.dma_start(rst_sb[:], r_dram.rearrange("o (c p) -> (o p) c", p=128))
                r_sb = smallp.tile([128, 2 * LT], F32, tag="rsb")
                nc.vector.reciprocal(r_sb[:], rst_sb[:])

                # ---- S = X1 @ X2^T and S^T, normalization folded ------
                # a2lhs[:, i, :] = r1-scaled S l-tile i   (lhsT for a2)
                # a1lhs[:, j, :] = r2-scaled S^T m-tile j (lhsT for a1)
                a2lhs = alhsp.tile([128, LT, L], F32R, tag="a2lhs")
                a1lhs = alhsp.tile([128, LT, L], F32R, tag="a1lhs")
                for i in range(LT):
                    s_ps = ps_s.tile([128, L], F32, tag="s")
                    for k in range(KT):
                        nc.tensor.matmul(s_ps[:], x1r[:, k, 128 * i:128 * (i + 1)],
                                         x2r[:, k, :], start=(k == 0), stop=(k == KT - 1))
                    nc.vector.tensor_scalar_mul(a2lhs[:, i, :], s_ps[:], r_sb[:, i:i + 1])
                for j in range(LT):
                    st_ps = ps_s.tile([128, L], F32, tag="s")
                    for k in range(KT):
                        nc.tensor.matmul(st_ps[:], x2r[:, k, 128 * j:128 * (j + 1)],
                                         x1r[:, k, :], start=(k == 0), stop=(k == KT - 1))
                    nc.vector.tensor_scalar_mul(a1lhs[:, j, :], st_ps[:],
                                                r_sb[:, LT + j:LT + j + 1])

                # ---- stage 2: a1 = (S D2^-1 W1) row-scaled by r1 ------
                import concourse.mybir as _mybir
                Copy = _mybir.ActivationFunctionType.Copy
                for i in range(LT):
                    a1_sb = aoutp.tile([128, D], F32, tag="aout")
                    for n in range(NT):
                        a1_ps = ps_a.tile([128, 512], F32, tag="a")
                        for jj in range(LT):
                            nc.tensor.matmul(
                                a1_ps[:], a1lhs[:, jj, 128 * i:128 * (i + 1)],
                                w1_sb[:, jj, 512 * n:512 * (n + 1)],
                                start=(jj == 0), stop=(jj == LT - 1))
                        nc.scalar.activation(a1_sb[:, 512 * n:512 * (n + 1)], a1_ps[:],
                                             Copy, scale=r_sb[:, i:i + 1])
                    nc.sync.dma_start(out1[b, 128 * i:128 * (i + 1), :], a1_sb[:])
                # ---- stage 2: a2 = (S^T D1^-1 W2) row-scaled by r2 ----
                for j in range(LT):
                    a2_sb = aoutp.tile([128, D], F32, tag="aout")
                    for n in range(NT):
                        a2_ps = ps_a.tile([128, 512], F32, tag="a")
                        for ii in range(LT):
                            nc.tensor.matmul(
                                a2_ps[:], a2lhs[:, ii, 128 * j:128 * (j + 1)],
                                w2_sb[:, ii, 512 * n:512 * (n + 1)],
                                start=(ii == 0), stop=(ii == LT - 1))
                        nc.vector.tensor_scalar_mul(a2_sb[:, 512 * n:512 * (n + 1)],
                                                    a2_ps[:], r_sb[:, LT + j:LT + j + 1])
                    nc.sync.dma_start(out2[b, 128 * j:128 * (j + 1), :], a2_sb[:])

    nc.compile()
    return nc


def _get_nc(bb=BB):
    if bb not in _CACHE:
        _CACHE[bb] = _build(bb)
    return _CACHE[bb]


def run_device(x1, x2, W1, W2, trace=False, bb=BB, n_batches=None):
    """Run the device part; returns (a1, a2) of shape (n, L, D) and the
    raw BassKernelResults (for exec_time_ns when trace=True)."""
    import concourse.bass_utils as bass_utils

    n = n_batches if n_batches is not None else bb * N_CORES
    x1 = np.ascontiguousarray(np.asarray(x1, dtype=np.float32).reshape(n, L, D)
                              .transpose(0, 2, 1))
    x2 = np.ascontiguousarray(np.asarray(x2, dtype=np.float32).reshape(n, L, D)
                              .transpose(0, 2, 1))
    W1 = np.ascontiguousarray(np.asarray(W1, dtype=np.float32))
    W2 = np.ascontiguousarray(np.asarray(W2, dtype=np.float32))

    nc = _get_nc(bb)
    in_maps = []
    for c in range(N_CORES):
        s = slice(c * bb, (c + 1) * bb)
        in_maps.append({"x1t": x1[s], "x2t": x2[s], "w1": W1, "w2": W2})
    res = bass_utils.run_bass_kernel_spmd(nc, in_maps, list(range(N_CORES)),
                                          trace=trace)
    a1 = np.concatenate([res.results[c]["out1"] for c in range(N_CORES)], axis=0)
    a2 = np.concatenate([res.results[c]["out2"] for c in range(N_CORES)], axis=0)
    return a1, a2, res


def kernel(x1, x2, W1, W2):
    x1 = np.asarray(x1, dtype=np.float32)
    x2 = np.asarray(x2, dtype=np.float32)
    a1, a2, _ = run_device(x1, x2, W1, W2, trace=False)
    attn1 = np.stack([x1.reshape(B, L, D), a1], axis=1)
    attn2 = np.stack([x2.reshape(B, L, D), a2], axis=1)
    return attn1, attn2
